# Initial kernel scaffold
#
"""DIN-style attention unit (dense MLP scorer + masked softmax + weighted sum)
on 8 Trainium2 NeuronCores, data-parallel over the batch dimension.

kernel(**inputs) takes the full unsharded inputs and returns (out, att):
  out: (2048, 128) f32, att: (2048, 200) f32
"""
import sys, types, os
sys.path.insert(0, "/opt/trn_rl_repo")
import numpy as np

# --- antenv.axon_hooks shim so run_bass_kernel_spmd(trace=True) can profile ---
try:
    import antenv.axon_hooks  # noqa: F401
except Exception:
    try:
        from trn_agent_boot.trn_boot import _ntff_profile_via_ctypes
        _hook = _ntff_profile_via_ctypes('/opt/axon/libaxon_pjrt.so')
    except Exception:
        _hook = None
    _m = types.ModuleType('antenv.axon_hooks')
    _m.get_axon_ntff_profile_hook = lambda: _hook
    _m.set_axon_ntff_profile_hook = lambda h: None
    sys.modules['antenv.axon_hooks'] = _m

import concourse.bass as bass
import concourse.tile as tile
from concourse import bacc, mybir
import concourse.bass_utils as bass_utils
from concourse import masks

bass_utils.upload_artifacts = lambda tmpdir: "local://" + str(tmpdir)

FP32 = mybir.dt.float32
BF16 = mybir.dt.bfloat16
U8 = mybir.dt.uint8
I32 = mybir.dt.int32
AF = mybir.ActivationFunctionType
ALU = mybir.AluOpType

N_CORES = 8
B, L, D, A = 2048, 200, 128, 64
BC = B // N_CORES          # 256 batch rows per core
L0, L1C = 128, L - 128     # l-chunk sizes: 128 + 72
FILLS = BC // 16           # 16 bank-fills of 16 batch rows
NEG_BIG = -1e30


def build_graph(alpha1: float, alpha2: float):
    nc = bacc.Bacc("TRN2", target_bir_lowering=False, debug=False,
                   num_devices=N_CORES)

    keys_d = nc.dram_tensor("keys", [BC, L, D], FP32, kind="ExternalInput").ap()
    query_d = nc.dram_tensor("query", [BC, D], FP32, kind="ExternalInput").ap()
    lens_d = nc.dram_tensor("lens", [BC], FP32, kind="ExternalInput").ap()
    W1_d = nc.dram_tensor("W1", [4 * D, A], FP32, kind="ExternalInput").ap()
    b1_d = nc.dram_tensor("b1", [A], FP32, kind="ExternalInput").ap()
    W2_d = nc.dram_tensor("W2", [A, A], FP32, kind="ExternalInput").ap()
    b2_d = nc.dram_tensor("b2", [A], FP32, kind="ExternalInput").ap()
    W3_d = nc.dram_tensor("W3", [A, 1], FP32, kind="ExternalInput").ap()
    out_d = nc.dram_tensor("out", [BC, D], FP32, kind="ExternalOutput").ap()
    att_d = nc.dram_tensor("att", [BC, L], FP32, kind="ExternalOutput").ap()

    with tile.TileContext(nc) as tc:
        import contextlib
        ctx = contextlib.ExitStack()
        with ctx:
            cst = ctx.enter_context(tc.tile_pool(name="cst", bufs=1))
            natf = ctx.enter_context(tc.tile_pool(name="natf", bufs=6))
            natb = ctx.enter_context(tc.tile_pool(name="natb", bufs=20))
            ktp = ctx.enter_context(tc.tile_pool(name="ktp", bufs=6))
            hp = ctx.enter_context(tc.tile_pool(name="hp", bufs=6))
            sxp = ctx.enter_context(tc.tile_pool(name="sxp", bufs=4))
            ps_t = ctx.enter_context(tc.tile_pool(name="ps_t", bufs=2, space="PSUM"))
            ps_1 = ctx.enter_context(tc.tile_pool(name="ps_1", bufs=2, space="PSUM"))
            ps_2 = ctx.enter_context(tc.tile_pool(name="ps_2", bufs=2, space="PSUM"))
            ps_s = ctx.enter_context(tc.tile_pool(name="ps_s", bufs=1, space="PSUM"))
            ps_o = ctx.enter_context(tc.tile_pool(name="ps_o", bufs=1, space="PSUM"))

            # ---------------- constants / weights prep ----------------
            ident = cst.tile([128, 128], BF16, tag="ident")
            masks.make_identity(nc, ident[:])
            identf = cst.tile([128, 128], FP32, tag="identf")
            masks.make_identity(nc, identf[:])

            iota_i = cst.tile([128, L], I32, tag="iota_i")
            nc.gpsimd.iota(iota_i[:], pattern=[[1, L]], base=0, channel_multiplier=0)
            iota_f = cst.tile([128, L], FP32, tag="iota_f")
            nc.vector.tensor_copy(iota_f[:], iota_i[:])
            negbig = cst.tile([128, L], FP32, tag="negbig")
            nc.gpsimd.memset(negbig[:], NEG_BIG)

            # W1 blocks: Wa=W1[0:D], Wb=[D:2D], Wc=[2D:3D], Wp=[3D:4D]
            W1v = W1_d.rearrange("(c d) a -> c d a", c=4)
            Wa = cst.tile([128, A], FP32, tag="Wa")
            nc.sync.dma_start(Wa[:], W1v[0])
            Wb = cst.tile([128, A], FP32, tag="Wb")
            nc.sync.dma_start(Wb[:], W1v[1])
            Wc = cst.tile([128, A], FP32, tag="Wc")
            nc.sync.dma_start(Wc[:], W1v[2])
            Wp = cst.tile([128, A], FP32, tag="Wp")
            nc.sync.dma_start(Wp[:], W1v[3])
            Wq_f = cst.tile([128, A], FP32, tag="Wq_f")
            nc.vector.tensor_tensor(Wq_f[:], Wa[:], Wc[:], op=ALU.add)
            Wq_bf = cst.tile([128, A], BF16, tag="Wq_bf")
            nc.vector.tensor_copy(Wq_bf[:], Wq_f[:])
            Wk_f = cst.tile([128, A], FP32, tag="Wk_f")
            nc.vector.tensor_tensor(Wk_f[:], Wb[:], Wc[:], op=ALU.subtract)

            # W2 replicated on both partition halves, bf16
            W2f = cst.tile([128, A], FP32, tag="W2f")
            nc.sync.dma_start(W2f[0:64, :], W2_d[:])
            nc.sync.dma_start(W2f[64:128, :], W2_d[:])
            W2bf = cst.tile([128, A], BF16, tag="W2bf")
            nc.vector.tensor_copy(W2bf[:], W2f[:])

            # W3 block-diagonal [128, 2]
            w3f = cst.tile([128, 2], FP32, tag="w3f")
            nc.gpsimd.memset(w3f[:], 0.0)
            nc.sync.dma_start(w3f[0:64, 0:1], W3_d[:])
            nc.sync.dma_start(w3f[64:128, 1:2], W3_d[:])
            w3diag = cst.tile([128, 2], BF16, tag="w3diag")
            nc.vector.tensor_copy(w3diag[:], w3f[:])

            # bias cols duplicated on both halves
            b1_dup = cst.tile([128, 1], FP32, tag="b1_dup")
            nc.sync.dma_start(b1_dup[0:64, :], b1_d[:, None])
            nc.sync.dma_start(b1_dup[64:128, :], b1_d[:, None])
            b2_dup = cst.tile([128, 1], FP32, tag="b2_dup")
            nc.sync.dma_start(b2_dup[0:64, :], b2_d[:, None])
            nc.sync.dma_start(b2_dup[64:128, :], b2_d[:, None])

            # lens slot columns [128, 2*FILLS]: col f*2+s row 32j+r = lens[16f+8s+2j+r]
            lens_sc = cst.tile([128, 2 * FILLS], FP32, tag="lens_sc")
            nc.gpsimd.memset(lens_sc[:], 1.0)
            lsrc = lens_d.rearrange("(f s j r) -> f s j r", s=2, j=4, r=2)
            # dst partition dims (j, r) strides (32, 1); free dims (f, s)
            ldst = lens_sc[:].rearrange("(j x r) (f s) -> j r f s x", j=4, r=2, s=2)[:, :, :, :, 0]
            nc.sync.dma_start(ldst.rearrange("j r f s -> f s j r"), lsrc)
            flag_sc = cst.tile([128, 2 * FILLS], FP32, tag="flag_sc")
            nc.vector.tensor_scalar(flag_sc[:], lens_sc[:], 0.0, None, op0=ALU.is_equal)

            # ---------------- per-group (128 b) prep: qT and bias1 ----------------
            # bias1_pairs[g]: [128, 64], column c = [Wq.T q_{2c}+b1 ; Wq.T q_{2c+1}+b1]
            qT_f_list, bias1_list = [], []
            for g in range(2):
                q_nat = sxp.tile([128, 128], FP32, tag="q_nat")
                nc.sync.dma_start(q_nat[:], query_d[g * 128:(g + 1) * 128, :])
                qT_ps = ps_t.tile([128, 128], FP32, tag="tps")
                nc.tensor.transpose(qT_ps[:], q_nat[:], identf[:])
                qT_f = cst.tile([128, 128], FP32, tag=f"qT_f{g}")
                nc.vector.tensor_copy(qT_f[:], qT_ps[:])
                qT_bf = cst.tile([128, 128], BF16, tag=f"qT_bf{g}")
                nc.vector.tensor_copy(qT_bf[:], qT_f[:])
                qT_f_list.append(qT_f)

                b1ps = ps_1.tile([128, 64], FP32, tag="l1ps")
                qT_pair = qT_bf[:].rearrange("p (c r) -> p c r", r=2)
                nc.tensor.matmul(b1ps[0:64, :], Wq_bf[:], qT_pair[:, :, 0],
                                 start=True, stop=True)
                nc.tensor.matmul(b1ps[64:128, :], Wq_bf[:], qT_pair[:, :, 1],
                                 start=True, stop=True, tile_position=(0, 64))
                bias1 = cst.tile([128, 64], FP32, tag=f"bias1_{g}")
                nc.scalar.activation(bias1[:], b1ps[:], AF.Identity, bias=b1_dup[:])
                bias1_list.append(bias1)

            # ---------------- main loop ----------------
            for f in range(FILLS):
                g = (f * 16) // 128
                sbank = ps_s.tile([128, 512], FP32, tag="sbank")
                obank = ps_o.tile([128, 512], FP32, tag="obank")
                nat_bf_pairs = []
                for p in range(8):
                    b0 = f * 16 + 2 * p
                    # -- load keys pair natural f32: [128, 512] --
                    nf = natf.tile([128, 512], FP32, tag="natf")
                    for h in range(2):
                        bb = b0 + h
                        nc.sync.dma_start(nf[0:128, 256 * h:256 * h + 128],
                                          keys_d[bb, 0:128, :])
                        nc.sync.dma_start(nf[0:L1C, 256 * h + 128:256 * h + 256],
                                          keys_d[bb, 128:L, :])
                    # -- cast to bf16 on gpsimd --
                    nb = natb.tile([128, 512], BF16, tag="natb")
                    nc.gpsimd.tensor_copy(nb[:], nf[:])
                    nat_bf_pairs.append(nb)
                    # -- PE transposes -> keysT pair [128, 400] bf16 --
                    tp = ps_t.tile([128, 400], BF16, tag="tps")
                    for h in range(2):
                        nc.tensor.transpose(tp[:, 200 * h:200 * h + 128],
                                            nb[0:128, 256 * h:256 * h + 128], ident[:])
                        nc.tensor.transpose(tp[:, 200 * h + 128:200 * h + 200],
                                            nb[0:L1C, 256 * h + 128:256 * h + 256],
                                            ident[0:L1C, 0:L1C])
                    kT = ktp.tile([128, 400], BF16, tag="kT")
                    nc.scalar.activation(kT[:], tp[:], AF.Copy)
                    # -- per-b L1 weights: Wkp = Wk + q*Wp (bf16) --
                    wkp = hp.tile([128, 128], BF16, tag="wkp")
                    qT_f = qT_f_list[g]
                    for h in range(2):
                        qcol = qT_f[:, (b0 + h - g * 128):(b0 + h - g * 128) + 1]
                        nc.vector.scalar_tensor_tensor(
                            wkp[:, 64 * h:64 * h + 64], Wp[:], qcol, Wk_f[:],
                            op0=ALU.mult, op1=ALU.add)
                    # -- L1: two col-packed matmuls -> psum1 [128, 200] --
                    psum1 = ps_1.tile([128, 200], FP32, tag="l1ps")
                    nc.tensor.matmul(psum1[0:64, :], wkp[:, 0:64], kT[:, 0:200],
                                     start=True, stop=True, tile_position=(0, 0))
                    nc.tensor.matmul(psum1[64:128, :], wkp[:, 64:128], kT[:, 200:400],
                                     start=True, stop=True, tile_position=(0, 64))
                    # -- PReLU1 (bias = qWq+b1) -> h1p bf16 --
                    h1p = hp.tile([128, 200], BF16, tag="h1p")
                    pcol = f * 8 + p - g * 64
                    nc.scalar.activation(h1p[:], psum1[:], AF.Prelu,
                                         bias=bias1_list[g][:, pcol:pcol + 1],
                                         alpha=alpha1)
                    # -- L2: row+col packed -> psum2 [128, 200] --
                    psum2 = ps_2.tile([128, 200], FP32, tag="l2ps")
                    nc.tensor.matmul(psum2[0:64, :], W2bf[0:64, :], h1p[0:64, :],
                                     start=True, stop=True, tile_position=(0, 0))
                    nc.tensor.matmul(psum2[64:128, :], W2bf[64:128, :], h1p[64:128, :],
                                     start=True, stop=True, tile_position=(64, 64))
                    # -- PReLU2 -> h2p bf16 --
                    h2p = hp.tile([128, 200], BF16, tag="h2p")
                    nc.scalar.activation(h2p[:], psum2[:], AF.Prelu,
                                         bias=b2_dup[:], alpha=alpha2)
                    # -- L3: block-diag W3 -> pair scores [2, 200] in slot --
                    j, s = p % 4, p // 4
                    nc.tensor.matmul(sbank[32 * j:32 * j + 2, 200 * s:200 * s + 200],
                                     w3diag[:], h2p[:], start=True, stop=True,
                                     tile_position=(0, 32 * j),
                                     skip_group_check=True)

                # ---------- softmax on the filled scores bank ----------
                e_sb = sxp.tile([128, 400], FP32, tag="e_sb")
                Zc = sxp.tile([128, 2], FP32, tag="Zc")
                rZc = sxp.tile([128, 2], FP32, tag="rZc")
                for s in range(2):
                    half = sbank[:, 200 * s:200 * s + 200]
                    minv = sxp.tile([128, L], U8, tag="minv")
                    nc.vector.tensor_scalar(minv[:], iota_f[:],
                                            lens_sc[:, 2 * f + s:2 * f + s + 1],
                                            None, op0=ALU.is_ge)
                    nc.vector.copy_predicated(half, minv[:], negbig[:])
                    eh = e_sb[:, 200 * s:200 * s + 200]
                    nc.scalar.activation(eh, half, AF.Exp,
                                         accum_out=Zc[:, s:s + 1])
                    nc.vector.tensor_scalar(eh, eh,
                                            flag_sc[:, 2 * f + s:2 * f + s + 1],
                                            None, op0=ALU.add)
                    nc.vector.scalar_tensor_tensor(Zc[:, s:s + 1],
                                                   flag_sc[:, 2 * f + s:2 * f + s + 1],
                                                   200.0, Zc[:, s:s + 1],
                                                   op0=ALU.mult, op1=ALU.add)
                    nc.vector.reciprocal(rZc[:, s:s + 1], Zc[:, s:s + 1])
                    nc.vector.tensor_scalar(eh, eh, rZc[:, s:s + 1], None,
                                            op0=ALU.mult)
                # att DMA out: b = 16f + 8s + 2j + r
                asrc = e_sb[:].rearrange("(j x r) (s l) -> j r s l x",
                                         j=4, r=2, s=2)[:, :, :, :, 0]
                nc.sync.dma_start(
                    att_d[f * 16:(f + 1) * 16, :].rearrange(
                        "(s j r) l -> j r s l", s=2, j=4),
                    asrc)
                # attT transposes (f32, junk columns included) -> bf16
                attT = sxp.tile([128, 512], BF16, tag="attT")  # cols: s*256+c*128.. c0 128 rows l, c1 72
                for s in range(2):
                    tp2 = ps_t.tile([128, 400], BF16, tag="tps")
                    # hmm: transpose out dtype must equal in dtype (f32) - use f32 psum then evac
                    tp2f = ps_t.tile([128, 200], FP32, tag="tpsf")
                    nc.tensor.transpose(tp2f[0:128, 0:128],
                                        e_sb[:, 200 * s:200 * s + 128], identf[:])
                    nc.tensor.transpose(tp2f[0:L1C, 128:200],
                                        e_sb[:, 200 * s + 128:200 * s + 200],
                                        identf[0:L1C, 0:L1C])
                    # evac: chunk0 [128 l, 128 b], chunk1 [72 l, 128 b]
                    nc.scalar.activation(attT[:, 256 * s:256 * s + 128],
                                         tp2f[:, 0:128], AF.Copy)
                    nc.scalar.activation(attT[0:L1C, 256 * s + 128:256 * s + 256],
                                         tp2f[0:L1C, 128:200].rearrange("p f -> p f")
                                         if False else tp2f[0:L1C, 128:200],
                                         AF.Copy)
                # wait: attT chunk layout wrong: transpose out [l, b-col]:
                # tp2f[:, 0:128] is [l=128, 128 bcols]? No: transpose in [128 rows(b-slots), 128 l]
                # -> out [128 l?? in partitions are b-slot rows, free l. out = in.T:
                # out partitions = l-chunk, free = 128 b-slot cols. OK as coded:
                # first transpose: in e_sb[:, s*200 : s*200+128] = [128 bslots, 128 l0]
                # -> out [128 l0, 128 bcols] at tp2f[:, 0:128]. correct.
                # second: in [128 bslots, 72 l1] -> out [72 l1, 128 bcols] -> tp2f[0:72, 128:256]?
                # coded tp2f[0:L1C, 128:200] has only 72 free cols - WRONG, needs 128.
                # (fixed below by using a [128, 256] psum tile)

                # ---------- output stage ----------
                for i in range(16):
                    bb = f * 16 + i
                    p_i, h_i = i // 2, i % 2
                    j, s = p_i % 4, p_i // 4
                    colb = 256 * s + 32 * j + h_i  # attT column for this b (chunk0)
                    nb = nat_bf_pairs[p_i]
                    jo, so = i % 4, i // 4
                    oslot = obank[32 * jo:32 * jo + 1, 128 * so:128 * so + 128]
                    nc.tensor.matmul(oslot, attT[0:128, colb:colb + 1],
                                     nb[0:128, 256 * h_i:256 * h_i + 128],
                                     start=True, stop=False,
                                     tile_position=(0, 32 * jo),
                                     skip_group_check=True)
                    nc.tensor.matmul(oslot, attT[0:L1C, colb + 128:colb + 129],
                                     nb[0:L1C, 256 * h_i + 128:256 * h_i + 256],
                                     start=False, stop=True,
                                     tile_position=(0, 32 * jo),
                                     skip_group_check=True)
                o_sb = sxp.tile([128, 512], FP32, tag="o_sb")
                nc.vector.tensor_copy(o_sb[:], obank[:])
                osrc = o_sb[:].rearrange("(j x) (s d) -> j s d x",
                                         j=4, s=4)[:, :, :, 0]
                nc.sync.dma_start(
                    out_d[f * 16:(f + 1) * 16, :].rearrange(
                        "(s j) d -> j s d", s=4, j=4),
                    osrc)

    nc.compile()
    return nc


_GRAPH_CACHE = {}


def kernel(**inputs):
    query = np.asarray(inputs["query"], np.float32)
    keys = np.asarray(inputs["keys"], np.float32)
    keys_length = np.asarray(inputs["keys_length"])
    W1 = np.asarray(inputs["W1"], np.float32)
    b1 = np.asarray(inputs["b1"], np.float32)
    a1 = float(np.asarray(inputs["a1"]).reshape(-1)[0])
    W2 = np.asarray(inputs["W2"], np.float32)
    b2 = np.asarray(inputs["b2"], np.float32)
    a2 = float(np.asarray(inputs["a2"]).reshape(-1)[0])
    W3 = np.asarray(inputs["W3"], np.float32)

    key = (a1, a2)
    if key not in _GRAPH_CACHE:
        _GRAPH_CACHE[key] = build_graph(a1, a2)
    nc = _GRAPH_CACHE[key]

    lens_f = keys_length.astype(np.float32)
    in_maps = []
    for c in range(N_CORES):
        sl = slice(c * BC, (c + 1) * BC)
        in_maps.append({
            "keys": np.ascontiguousarray(keys[sl]),
            "query": np.ascontiguousarray(query[sl]),
            "lens": np.ascontiguousarray(lens_f[sl]),
            "W1": W1, "b1": b1, "W2": W2, "b2": b2, "W3": W3,
        })
    res = bass_utils.run_bass_kernel_spmd(
        nc, in_maps, core_ids=list(range(N_CORES)),
        trace=bool(int(os.environ.get("KERNEL_TRACE", "0"))))
    kernel.last_exec_time_ns = res.exec_time_ns
    out = np.concatenate([res.results[c]["out"] for c in range(N_CORES)], 0)
    att = np.concatenate([res.results[c]["att"] for c in range(N_CORES)], 0)
    return out, att


# revision 6
# speedup vs baseline: 1.3148x; 1.3148x over previous
"""DIN-style attention unit (dense MLP scorer + masked softmax + weighted sum)
on 8 Trainium2 NeuronCores, data-parallel over the batch dimension.

kernel(**inputs) takes the full unsharded inputs and returns (out, att):
  out: (2048, 128) f32, att: (2048, 200) f32
"""
import sys, types, os
sys.path.insert(0, "/opt/trn_rl_repo")
import numpy as np

# --- antenv.axon_hooks shim so run_bass_kernel_spmd(trace=True) can profile ---
try:
    import antenv.axon_hooks  # noqa: F401
except Exception:
    try:
        from trn_agent_boot.trn_boot import _ntff_profile_via_ctypes
        _hook = _ntff_profile_via_ctypes('/opt/axon/libaxon_pjrt.so')
    except Exception:
        _hook = None
    _m = types.ModuleType('antenv.axon_hooks')
    _m.get_axon_ntff_profile_hook = lambda: _hook
    _m.set_axon_ntff_profile_hook = lambda h: None
    sys.modules['antenv.axon_hooks'] = _m

import concourse.bass as bass
import concourse.tile as tile
from concourse import bacc, mybir
import concourse.bass_utils as bass_utils
from concourse import masks

bass_utils.upload_artifacts = lambda tmpdir: "local://" + str(tmpdir)

FP32 = mybir.dt.float32
BF16 = mybir.dt.bfloat16
U8 = mybir.dt.uint8
I32 = mybir.dt.int32
AF = mybir.ActivationFunctionType
ALU = mybir.AluOpType

N_CORES = 8
B, L, D, A = 2048, 200, 128, 64
BC = B // N_CORES          # 256 batch rows per core
L0, L1C = 128, L - 128     # l-chunk sizes: 128 + 72
FILLS = BC // 16           # 16 bank-fills of 16 batch rows
NEG_BIG = -1e30


def build_graph(alpha1: float, alpha2: float):
    nc = bacc.Bacc("TRN2", target_bir_lowering=False, debug=False,
                   num_devices=N_CORES)

    keys_d = nc.dram_tensor("keys", [BC, L, D], FP32, kind="ExternalInput").ap()
    query_d = nc.dram_tensor("query", [BC, D], FP32, kind="ExternalInput").ap()
    lens_d = nc.dram_tensor("lens", [BC], FP32, kind="ExternalInput").ap()
    W1_d = nc.dram_tensor("W1", [4 * D, A], FP32, kind="ExternalInput").ap()
    b1_d = nc.dram_tensor("b1", [A], FP32, kind="ExternalInput").ap()
    W2_d = nc.dram_tensor("W2", [A, A], FP32, kind="ExternalInput").ap()
    b2_d = nc.dram_tensor("b2", [A], FP32, kind="ExternalInput").ap()
    W3_d = nc.dram_tensor("W3", [A, 1], FP32, kind="ExternalInput").ap()
    out_d = nc.dram_tensor("out", [BC, D], FP32, kind="ExternalOutput").ap()
    att_d = nc.dram_tensor("att", [BC, L], FP32, kind="ExternalOutput").ap()

    with tile.TileContext(nc) as tc:
        import contextlib
        ctx = contextlib.ExitStack()
        with ctx:
            cst = ctx.enter_context(tc.tile_pool(name="cst", bufs=1))
            natf = ctx.enter_context(tc.tile_pool(name="natf", bufs=6))
            natb = ctx.enter_context(tc.tile_pool(name="natb", bufs=20))
            ktp = ctx.enter_context(tc.tile_pool(name="ktp", bufs=6))
            hp = ctx.enter_context(tc.tile_pool(name="hp", bufs=6))
            sxp = ctx.enter_context(tc.tile_pool(name="sxp", bufs=4))
            ps_t = ctx.enter_context(tc.tile_pool(name="ps_t", bufs=2, space="PSUM"))
            ps_x = ctx.enter_context(tc.tile_pool(name="ps_x", bufs=1, space="PSUM"))
            ps_12 = ctx.enter_context(tc.tile_pool(name="ps_12", bufs=3, space="PSUM"))
            ps_s = ctx.enter_context(tc.tile_pool(name="ps_s", bufs=1, space="PSUM"))
            ps_o = ctx.enter_context(tc.tile_pool(name="ps_o", bufs=1, space="PSUM"))

            # ---------------- constants / weights prep ----------------
            ident = cst.tile([128, 128], BF16, tag="ident")
            masks.make_identity(nc, ident[:])
            identf = cst.tile([128, 128], FP32, tag="identf")
            masks.make_identity(nc, identf[:])

            iota_i = cst.tile([128, L], I32, tag="iota_i")
            nc.gpsimd.iota(iota_i[:], pattern=[[1, L]], base=0, channel_multiplier=0)
            iota_f = cst.tile([128, L], FP32, tag="iota_f")
            nc.vector.tensor_copy(iota_f[:], iota_i[:])
            negbig = cst.tile([128, L], FP32, tag="negbig")
            nc.gpsimd.memset(negbig[:], NEG_BIG)

            # W1 blocks: Wa=W1[0:D], Wb=[D:2D], Wc=[2D:3D], Wp=[3D:4D]
            W1v = W1_d.rearrange("(c d) a -> c d a", c=4)
            Wa = cst.tile([128, A], FP32, tag="Wa")
            nc.sync.dma_start(Wa[:], W1v[0])
            Wb = cst.tile([128, A], FP32, tag="Wb")
            nc.sync.dma_start(Wb[:], W1v[1])
            Wc = cst.tile([128, A], FP32, tag="Wc")
            nc.sync.dma_start(Wc[:], W1v[2])
            Wp = cst.tile([128, A], FP32, tag="Wp")
            nc.sync.dma_start(Wp[:], W1v[3])
            Wq_f = cst.tile([128, A], FP32, tag="Wq_f")
            nc.vector.tensor_tensor(Wq_f[:], Wa[:], Wc[:], op=ALU.add)
            Wq_bf = cst.tile([128, A], BF16, tag="Wq_bf")
            nc.vector.tensor_copy(Wq_bf[:], Wq_f[:])
            Wk_f = cst.tile([128, A], FP32, tag="Wk_f")
            nc.vector.tensor_tensor(Wk_f[:], Wb[:], Wc[:], op=ALU.subtract)

            # W2 replicated on both partition halves, bf16
            W2f = cst.tile([128, A], FP32, tag="W2f")
            nc.sync.dma_start(W2f[0:64, :], W2_d[:])
            nc.sync.dma_start(W2f[64:128, :], W2_d[:])
            W2bf = cst.tile([128, A], BF16, tag="W2bf")
            nc.vector.tensor_copy(W2bf[:], W2f[:])

            # W3 block-diagonal [128, 2]
            w3f = cst.tile([128, 2], FP32, tag="w3f")
            nc.gpsimd.memset(w3f[:], 0.0)
            nc.sync.dma_start(w3f[0:64, 0:1], W3_d[:])
            nc.sync.dma_start(w3f[64:128, 1:2], W3_d[:])
            w3diag = cst.tile([128, 2], BF16, tag="w3diag")
            nc.vector.tensor_copy(w3diag[:], w3f[:])

            # bias cols duplicated on both halves
            b1_dup = cst.tile([128, 1], FP32, tag="b1_dup")
            nc.sync.dma_start(b1_dup[0:64, :], b1_d[:, None])
            nc.sync.dma_start(b1_dup[64:128, :], b1_d[:, None])
            b2_dup = cst.tile([128, 1], FP32, tag="b2_dup")
            nc.sync.dma_start(b2_dup[0:64, :], b2_d[:, None])
            nc.sync.dma_start(b2_dup[64:128, :], b2_d[:, None])

            # lens slot columns [128, 2*FILLS]: col f*2+s row 32j+r = lens[16f+8s+2j+r]
            lens_sc = cst.tile([128, 2 * FILLS], FP32, tag="lens_sc")
            nc.gpsimd.memset(lens_sc[:], 1.0)
            lsrc = lens_d.rearrange("(f s j r) -> f s j r", s=2, j=4, r=2)
            ldst = lens_sc[:].rearrange("(j x r) (f s) -> j r f s x",
                                        j=4, r=2, s=2)[:, :, :, :, 0]
            for s in range(2):
                for j in range(4):
                    nc.sync.dma_start(
                        ldst[j, :, :, s],
                        lsrc[:, s, j].rearrange("f r -> r f"))
            flag_sc = cst.tile([128, 2 * FILLS], FP32, tag="flag_sc")
            nc.vector.tensor_scalar(flag_sc[:], lens_sc[:], 0.0, None, op0=ALU.is_equal)

            # ---------------- per-group (128 b) prep: qT and bias1 ----------------
            # bias1_pairs[g]: [128, 64], column c = [Wq.T q_{2c}+b1 ; Wq.T q_{2c+1}+b1]
            qT_f_list, bias1_list = [], []
            for g in range(2):
                q_nat = sxp.tile([128, 128], FP32, tag="q_nat")
                nc.sync.dma_start(q_nat[:], query_d[g * 128:(g + 1) * 128, :])
                qT_ps = ps_t.tile([128, 128], FP32, tag="tps")
                nc.tensor.transpose(qT_ps[:], q_nat[:], identf[:])
                qT_f = cst.tile([128, 128], FP32, tag=f"qT_f{g}")
                nc.vector.tensor_copy(qT_f[:], qT_ps[:])
                qT_bf = cst.tile([128, 128], BF16, tag=f"qT_bf{g}")
                nc.vector.tensor_copy(qT_bf[:], qT_f[:])
                qT_f_list.append(qT_f)

                b1ps = ps_12.tile([128, 64], FP32, tag="l12")
                qT_pair = qT_bf[:].rearrange("p (c r) -> p c r", r=2)
                nc.tensor.matmul(b1ps[0:64, :], Wq_bf[:], qT_pair[:, :, 0],
                                 start=True, stop=True)
                nc.tensor.matmul(b1ps[64:128, :], Wq_bf[:], qT_pair[:, :, 1],
                                 start=True, stop=True, tile_position=(0, 64))
                bias1 = cst.tile([128, 64], FP32, tag=f"bias1_{g}")
                nc.scalar.activation(bias1[:], b1ps[:], AF.Identity, bias=b1_dup[:])
                bias1_list.append(bias1)

            # ---------------- main loop ----------------
            for f in range(FILLS):
                g = (f * 16) // 128
                sbank = ps_s.tile([128, 512], FP32, tag="sbank")
                obank = ps_o.tile([128, 512], FP32, tag="obank")
                if f == 0:
                    nc.vector.memset(sbank[:], 0.0)
                    nc.vector.memset(obank[:], 0.0)
                nat_bf_pairs = []
                for p in range(8):
                    b0 = f * 16 + 2 * p
                    # -- load keys pair natural f32: [128, 512] --
                    nf = natf.tile([128, 512], FP32, tag="natf")
                    for h in range(2):
                        bb = b0 + h
                        nc.sync.dma_start(nf[0:128, 256 * h:256 * h + 128],
                                          keys_d[bb, 0:128, :])
                        nc.sync.dma_start(nf[0:L1C, 256 * h + 128:256 * h + 256],
                                          keys_d[bb, 128:L, :])
                    # -- cast to bf16 on gpsimd --
                    nb = natb.tile([128, 512], BF16, tag="natb")
                    nc.gpsimd.tensor_copy(nb[:], nf[:])
                    nat_bf_pairs.append(nb)
                    # -- PE transposes -> keysT pair [128, 400] bf16 --
                    tp = ps_t.tile([128, 400], BF16, tag="tps")
                    for h in range(2):
                        nc.tensor.transpose(tp[:, 200 * h:200 * h + 128],
                                            nb[0:128, 256 * h:256 * h + 128], ident[:])
                        nc.tensor.transpose(tp[:, 200 * h + 128:200 * h + 200],
                                            nb[0:L1C, 256 * h + 128:256 * h + 256],
                                            ident[0:L1C, 0:L1C])
                    kT = ktp.tile([128, 400], BF16, tag="kT")
                    nc.scalar.activation(kT[:], tp[:], AF.Copy)
                    # -- per-b L1 weights: Wkp = Wk + q*Wp (bf16) --
                    wkp = hp.tile([128, 128], BF16, tag="wkp")
                    qT_f = qT_f_list[g]
                    for h in range(2):
                        qcol = qT_f[:, (b0 + h - g * 128):(b0 + h - g * 128) + 1]
                        nc.vector.scalar_tensor_tensor(
                            wkp[:, 64 * h:64 * h + 64], Wp[:], qcol, Wk_f[:],
                            op0=ALU.mult, op1=ALU.add)
                    # -- L1: two col-packed matmuls -> psum1 [128, 200] --
                    psum1 = ps_12.tile([128, 200], FP32, tag="l12")
                    nc.tensor.matmul(psum1[0:64, :], wkp[:, 0:64], kT[:, 0:200],
                                     start=True, stop=True, tile_position=(0, 0))
                    nc.tensor.matmul(psum1[64:128, :], wkp[:, 64:128], kT[:, 200:400],
                                     start=True, stop=True, tile_position=(0, 64))
                    # -- PReLU1 (bias = qWq+b1) -> h1p bf16 --
                    h1p = hp.tile([128, 200], BF16, tag="h1p")
                    pcol = f * 8 + p - g * 64
                    nc.scalar.activation(h1p[:], psum1[:], AF.Prelu,
                                         bias=bias1_list[g][:, pcol:pcol + 1],
                                         alpha=alpha1)
                    # -- L2: row+col packed -> psum2 [128, 200] --
                    psum2 = ps_12.tile([128, 200], FP32, tag="l12")
                    nc.tensor.matmul(psum2[0:64, :], W2bf[0:64, :], h1p[0:64, :],
                                     start=True, stop=True, tile_position=(0, 0))
                    nc.tensor.matmul(psum2[64:128, :], W2bf[64:128, :], h1p[64:128, :],
                                     start=True, stop=True, tile_position=(64, 64))
                    # -- PReLU2 -> h2p bf16 --
                    h2p = hp.tile([128, 200], BF16, tag="h2p")
                    nc.scalar.activation(h2p[:], psum2[:], AF.Prelu,
                                         bias=b2_dup[:], alpha=alpha2)
                    # -- L3: block-diag W3 -> pair scores [2, 200] in slot --
                    j, s = p % 4, p // 4
                    nc.tensor.matmul(sbank[32 * j:32 * j + 2, 200 * s:200 * s + 200],
                                     w3diag[:], h2p[:], start=True, stop=True,
                                     tile_position=(0, 32 * j),
                                     skip_group_check=True)

                # ---------- softmax on the filled scores bank ----------
                e_sb = sxp.tile([128, 400], FP32, tag="e_sb")
                Zc = sxp.tile([128, 2], FP32, tag="Zc")
                rZc = sxp.tile([128, 2], FP32, tag="rZc")
                for s in range(2):
                    half = sbank[:, 200 * s:200 * s + 200]
                    minv = sxp.tile([128, L], U8, tag="minv")
                    nc.vector.tensor_scalar(minv[:], iota_f[:],
                                            lens_sc[:, 2 * f + s:2 * f + s + 1],
                                            None, op0=ALU.is_ge)
                    nc.vector.copy_predicated(half, minv[:], negbig[:])
                    eh = e_sb[:, 200 * s:200 * s + 200]
                    nc.scalar.activation(eh, half, AF.Exp,
                                         accum_out=Zc[:, s:s + 1])
                    nc.vector.tensor_scalar(eh, eh,
                                            flag_sc[:, 2 * f + s:2 * f + s + 1],
                                            None, op0=ALU.add)
                    nc.vector.scalar_tensor_tensor(Zc[:, s:s + 1],
                                                   flag_sc[:, 2 * f + s:2 * f + s + 1],
                                                   200.0, Zc[:, s:s + 1],
                                                   op0=ALU.mult, op1=ALU.add)
                    nc.vector.reciprocal(rZc[:, s:s + 1], Zc[:, s:s + 1])
                    nc.vector.tensor_scalar(eh, eh, rZc[:, s:s + 1], None,
                                            op0=ALU.mult)
                # att DMA out: b = 16f + 8s + 2j + r
                asrc = e_sb[:].rearrange("(j x r) (s l) -> j r s l x",
                                         j=4, r=2, s=2)[:, :, :, :, 0]
                nc.sync.dma_start(
                    att_d[f * 16:(f + 1) * 16, :].rearrange(
                        "(s j r) l -> j r s l", s=2, j=4),
                    asrc)
                # attT transposes (f32, junk columns included) -> bf16.
                # attT cols: 256*s + chunk*128 + (32j + r); chunk0 rows l=0..127,
                # chunk1 rows l=0..71 (l-128).
                attT = sxp.tile([128, 512], BF16, tag="attT")
                for s in range(2):
                    tp2f = ps_x.tile([128, 256], FP32, tag="tpsf")
                    nc.tensor.transpose(tp2f[0:128, 0:128],
                                        e_sb[:, 200 * s:200 * s + 128], identf[:])
                    nc.tensor.transpose(tp2f[0:L1C, 128:256],
                                        e_sb[:, 200 * s + 128:200 * s + 200],
                                        identf[:])
                    nc.scalar.activation(attT[:, 256 * s:256 * s + 128],
                                         tp2f[:, 0:128], AF.Copy)
                    nc.scalar.activation(attT[0:L1C, 256 * s + 128:256 * s + 256],
                                         tp2f[0:L1C, 128:256], AF.Copy)

                # ---------- output stage ----------
                for i in range(16):
                    bb = f * 16 + i
                    p_i, h_i = i // 2, i % 2
                    j, s = p_i % 4, p_i // 4
                    colb = 256 * s + 32 * j + h_i  # attT column for this b (chunk0)
                    nb = nat_bf_pairs[p_i]
                    jo, so = i % 4, i // 4
                    oslot = obank[32 * jo:32 * jo + 1, 128 * so:128 * so + 128]
                    nc.tensor.matmul(oslot, attT[0:128, colb:colb + 1],
                                     nb[0:128, 256 * h_i:256 * h_i + 128],
                                     start=True, stop=False,
                                     tile_position=(0, 32 * jo),
                                     skip_group_check=True)
                    nc.tensor.matmul(oslot, attT[0:L1C, colb + 128:colb + 129],
                                     nb[0:L1C, 256 * h_i + 128:256 * h_i + 256],
                                     start=False, stop=True,
                                     tile_position=(0, 32 * jo),
                                     skip_group_check=True)
                o_sb = sxp.tile([128, 512], FP32, tag="o_sb")
                nc.vector.tensor_copy(o_sb[:], obank[:])
                osrc = o_sb[:].rearrange("(j x) (s d) -> j s d x",
                                         j=4, s=4)[:, :, :, 0]
                nc.sync.dma_start(
                    out_d[f * 16:(f + 1) * 16, :].rearrange(
                        "(s j) d -> j s d", s=4, j=4),
                    osrc)

    nc.compile()
    return nc


_GRAPH_CACHE = {}


def kernel(**inputs):
    query = np.asarray(inputs["query"], np.float32)
    keys = np.asarray(inputs["keys"], np.float32)
    keys_length = np.asarray(inputs["keys_length"])
    W1 = np.asarray(inputs["W1"], np.float32)
    b1 = np.asarray(inputs["b1"], np.float32)
    a1 = float(np.asarray(inputs["a1"]).reshape(-1)[0])
    W2 = np.asarray(inputs["W2"], np.float32)
    b2 = np.asarray(inputs["b2"], np.float32)
    a2 = float(np.asarray(inputs["a2"]).reshape(-1)[0])
    W3 = np.asarray(inputs["W3"], np.float32)

    key = (a1, a2)
    if key not in _GRAPH_CACHE:
        _GRAPH_CACHE[key] = build_graph(a1, a2)
    nc = _GRAPH_CACHE[key]

    lens_f = keys_length.astype(np.float32)
    in_maps = []
    for c in range(N_CORES):
        sl = slice(c * BC, (c + 1) * BC)
        in_maps.append({
            "keys": np.ascontiguousarray(keys[sl]),
            "query": np.ascontiguousarray(query[sl]),
            "lens": np.ascontiguousarray(lens_f[sl]),
            "W1": W1, "b1": b1, "W2": W2, "b2": b2, "W3": W3,
        })
    res = bass_utils.run_bass_kernel_spmd(
        nc, in_maps, core_ids=list(range(N_CORES)),
        trace=bool(int(os.environ.get("KERNEL_TRACE", "0"))))
    kernel.last_exec_time_ns = res.exec_time_ns
    out = np.concatenate([res.results[c]["out"] for c in range(N_CORES)], 0)
    att = np.concatenate([res.results[c]["att"] for c in range(N_CORES)], 0)
    return out, att
